# revision 5
# baseline (speedup 1.0000x reference)
"""Trainium2 Bass kernel for nn_Attention_75093208203309 (sparse attention).

Contract: kernel(**inputs) takes FULL unsharded inputs (numpy), returns the
FULL [4096, 1024] float32 output. Internally shards query rows across 8
NeuronCores; k/v are computed locally per-core and all-gathered on-device.

Layout strategy (all transposes done host-side in numpy):
  - Per core i (rows = 512*i .. 512*(i+1)):
      qT, qcT   [D, 512]   computed on device from xT shard (f32r matmuls)
      kT_local  [D, 512] -> AllGather -> zk  (kT of all rows)
      v_local   [512, D] -> AllGather -> zv  (v natural layout)
      S.T tiles [mk=128, m=512] = k @ qT : lhsT = kT slice (shared stationary
        operand with conn.T = k @ qcT), rhs = qT / qcT.
      mask term added via identity matmul into PSUM; exp on ACT with fixed
        -10000 shift folded into host-scaled masks; softmax normalization
        applied after O = E @ v using per-partition reciprocal sums.
"""

import numpy as np

import concourse.bacc as bacc
import concourse.mybir as mybir
import concourse.tile as tile
from concourse import bass_utils

f32 = mybir.dt.float32
f32r = mybir.dt.float32r
AF = mybir.ActivationFunctionType
ALU = mybir.AluOpType

NCORES = 8
N, D = 4096, 1024
M = N // NCORES          # 512 rows per core
MT = M // 128            # 4 m-tiles
G = N // 128             # 32 mk-tiles
DC = D // 128            # 8 contraction tiles
MSCALE = 320000.0        # 10000 * 32 (folded softmax scale 1/sqrt(D)=1/32)
RG = [list(range(NCORES))]


def build(bias_val: float):
    nc = bacc.Bacc(None, num_devices=NCORES, debug=False)

    xt = nc.dram_tensor("xt", [DC, 128, M], f32, kind="ExternalInput")
    xn = nc.dram_tensor("xn", [MT, 128, D], f32, kind="ExternalInput")
    wqt = nc.dram_tensor("wqt", [DC, 128, D], f32, kind="ExternalInput")
    wkt = nc.dram_tensor("wkt", [DC, 128, D], f32, kind="ExternalInput")
    wvt = nc.dram_tensor("wvt", [DC, 128, D], f32, kind="ExternalInput")
    cn = nc.dram_tensor("cn", [DC, 128, D], f32, kind="ExternalInput")
    bq = nc.dram_tensor("bq", [128, DC], f32, kind="ExternalInput")
    bk = nc.dram_tensor("bk", [128, DC], f32, kind="ExternalInput")
    bv = nc.dram_tensor("bv", [1, D], f32, kind="ExternalInput")
    bnd = nc.dram_tensor("bnd", [MT, 128, 1], f32, kind="ExternalInput")
    amh = nc.dram_tensor("amh", [G, 128, M], f32, kind="ExternalInput")
    lmh = nc.dram_tensor("lmh", [G, 128, M], f32, kind="ExternalInput")
    ident = nc.dram_tensor("ident", [128, 128], f32, kind="ExternalInput")
    ones8 = nc.dram_tensor("ones8", [128, 8], f32, kind="ExternalInput")
    ones1 = nc.dram_tensor("ones1", [1, 128], f32, kind="ExternalInput")
    out = nc.dram_tensor("out", [MT, 128, D], f32, kind="ExternalOutput")

    with tile.TileContext(nc) as tc:
        with (
            tc.tile_pool(name="persist", bufs=1) as pp,
            tc.tile_pool(name="dram", bufs=1, space="DRAM") as dp,
        ):
            E3 = pp.tile([128, G, M], f32r, name="E3")
            id_s = pp.tile([128, 128], f32r, name="id_s")
            ones_s = pp.tile([128, 8], f32r, name="ones_s")
            onesk1 = pp.tile([1, 128], f32r, name="onesk1")
            bq_s = pp.tile([128, DC], f32, name="bq_s")
            bk_s = pp.tile([128, DC], f32, name="bk_s")
            bv_s = pp.tile([1, D], f32r, name="bv_s")
            bnd_s = pp.tile([128, MT], f32, name="bnd_s")
            recip_s = pp.tile([128, MT], f32, name="recip_s")
            s1_s = pp.tile([128, MT], f32, name="s1_s")
            omb_s = pp.tile([128, MT], f32, name="omb_s")

            nc.sync.dma_start(id_s[:], ident.ap().bitcast(f32r))
            nc.sync.dma_start(ones_s[:], ones8.ap().bitcast(f32r))
            nc.sync.dma_start(onesk1[:], ones1.ap().bitcast(f32r))
            nc.sync.dma_start(bq_s[:], bq.ap())
            nc.sync.dma_start(bk_s[:], bk.ap())
            nc.sync.dma_start(bv_s[:], bv.ap().bitcast(f32r))
            for mt in range(MT):
                nc.sync.dma_start(bnd_s[:, mt : mt + 1], bnd.ap()[mt])
            nc.vector.tensor_scalar(
                omb_s[:], bnd_s[:], -1.0, 1.0, ALU.mult, ALU.add
            )

            kt_loc = dp.tile([DC, 128, M], f32, name="kt_loc")
            v_loc = dp.tile([MT, 128, D], f32, name="v_loc")
            zk = dp.tile([NCORES, DC, 128, M], f32, name="zk")
            zv = dp.tile([NCORES, MT, 128, D], f32, name="zv")

            with tc.tile_pool(name="qpool", bufs=1) as qp:
                qt_s = qp.tile([128, DC, M], f32r, name="qt_s")
                qct_s = qp.tile([128, DC, M], f32r, name="qct_s")

                # ---------------- QKV projections ----------------
                with (
                    tc.tile_pool(name="qkv_w", bufs=2) as wp,
                    tc.tile_pool(name="qkv_x", bufs=1) as xp,
                    tc.tile_pool(name="qkv_sb", bufs=3) as sp,
                    tc.tile_pool(name="qkv_ps", bufs=4, space="PSUM") as ps1,
                ):
                    xt_s = xp.tile([128, DC, M], f32r, name="xt_s")
                    nc.sync.dma_start(
                        xt_s[:], xt.ap().rearrange("t p m -> p t m").bitcast(f32r)
                    )

                    # kT first: it feeds the first all-gather.
                    wk_s = wp.tile([128, DC, D], f32r, tag="w", name="wk_s")
                    nc.sync.dma_start(
                        wk_s[:], wkt.ap().rearrange("t p o -> p t o").bitcast(f32r)
                    )
                    for ot in range(DC):
                        p = ps1.tile([128, M], f32, tag="ps1", name="kps")
                        for t in range(DC):
                            nc.tensor.matmul(
                                p[:],
                                wk_s[:, t, ot * 128 : (ot + 1) * 128],
                                xt_s[:, t, :],
                                start=(t == 0),
                                stop=(t == DC - 1),
                            )
                        kt_sb = sp.tile([128, M], f32, tag="kvsb", name="kt_sb")
                        nc.scalar.activation(
                            kt_sb[:], p[:], AF.Identity, bias=bk_s[:, ot : ot + 1]
                        )
                        nc.sync.dma_start(kt_loc[ot], kt_sb[:])
                    nc.gpsimd.collective_compute(
                        "AllGather", ALU.bypass, replica_groups=RG,
                        ins=[kt_loc[:].opt()], outs=[zk[:].opt()],
                    )

                    wv_s = wp.tile([128, DC, D], f32r, tag="w", name="wv_s")
                    nc.sync.dma_start(
                        wv_s[:], wvt.ap().rearrange("t p o -> p t o").bitcast(f32r)
                    )
                    for mt in range(MT):
                        for dh in range(2):
                            p = ps1.tile([128, 512], f32, tag="ps1", name="vps")
                            for t in range(DC):
                                nc.tensor.matmul(
                                    p[:],
                                    xt_s[:, t, mt * 128 : (mt + 1) * 128],
                                    wv_s[:, t, dh * 512 : (dh + 1) * 512],
                                    start=(t == 0),
                                    stop=False,
                                )
                            nc.tensor.matmul(
                                p[:],
                                onesk1[:, :],
                                bv_s[:, dh * 512 : (dh + 1) * 512],
                                start=False,
                                stop=True,
                            )
                            v_sb = sp.tile([128, 512], f32, tag="kvsb", name="v_sb")
                            nc.scalar.copy(v_sb[:], p[:])
                            nc.sync.dma_start(
                                v_loc[mt, :, dh * 512 : (dh + 1) * 512], v_sb[:]
                            )
                    nc.gpsimd.collective_compute(
                        "AllGather", ALU.bypass, replica_groups=RG,
                        ins=[v_loc[:].opt()], outs=[zv[:].opt()],
                    )

                    wq_s = wp.tile([128, DC, D], f32r, tag="w", name="wq_s")
                    nc.sync.dma_start(
                        wq_s[:], wqt.ap().rearrange("t p o -> p t o").bitcast(f32r)
                    )
                    for ot in range(DC):
                        p = ps1.tile([128, M], f32, tag="ps1", name="qps")
                        for t in range(DC):
                            nc.tensor.matmul(
                                p[:],
                                wq_s[:, t, ot * 128 : (ot + 1) * 128],
                                xt_s[:, t, :],
                                start=(t == 0),
                                stop=(t == DC - 1),
                            )
                        nc.scalar.activation(
                            qt_s[:, ot, :], p[:], AF.Identity,
                            bias=bq_s[:, ot : ot + 1],
                        )

                    cn_s = wp.tile([128, DC, D], f32r, tag="w", name="cn_s")
                    nc.sync.dma_start(
                        cn_s[:], cn.ap().rearrange("t p o -> p t o").bitcast(f32r)
                    )
                    for ot in range(DC):
                        p = ps1.tile([128, M], f32, tag="ps1", name="cps")
                        for t in range(DC):
                            nc.tensor.matmul(
                                p[:],
                                cn_s[:, t, ot * 128 : (ot + 1) * 128],
                                qt_s[:, t, :],
                                start=(t == 0),
                                stop=(t == DC - 1),
                            )
                        nc.scalar.copy(qct_s[:, ot, :], p[:])

                # ---------------- S phase: logits, mask, exp ----------------
                with (
                    tc.tile_pool(name="s_kt", bufs=2) as kp,
                    tc.tile_pool(name="s_m", bufs=4) as mp,
                    tc.tile_pool(name="s_t", bufs=3) as tpool,
                    tc.tile_pool(name="s_psA", bufs=2, space="PSUM") as psA,
                    tc.tile_pool(name="s_psB", bufs=3, space="PSUM") as psB,
                ):

                    def flush_s(prev):
                        B_prev, t3_prev, g_prev = prev
                        nc.tensor.matmul(
                            B_prev[:], id_s[:], t3_prev[:], start=False, stop=True
                        )
                        nc.scalar.activation(
                            E3[:, g_prev, :], B_prev[:], AF.Exp, scale=1.0 / 32.0
                        )

                    prev = None
                    for j in range(NCORES):
                        ktb = kp.tile([128, DC, M], f32r, tag="kt", name="ktb")
                        nc.sync.dma_start(
                            ktb[:],
                            zk[j].rearrange("t p m -> p t m").bitcast(f32r),
                        )
                        for gi in range(4):
                            g = j * 4 + gi
                            B = psB.tile([128, M], f32, tag="B", name="Bps")
                            for t in range(DC):
                                nc.tensor.matmul(
                                    B[:],
                                    ktb[:, t, gi * 128 : (gi + 1) * 128],
                                    qt_s[:, t, :],
                                    start=(t == 0),
                                    stop=False,
                                )
                            A = psA.tile([128, M], f32, tag="A", name="Aps")
                            for t in range(DC):
                                nc.tensor.matmul(
                                    A[:],
                                    ktb[:, t, gi * 128 : (gi + 1) * 128],
                                    qct_s[:, t, :],
                                    start=(t == 0),
                                    stop=(t == DC - 1),
                                )
                            if prev is not None:
                                flush_s(prev)
                            am_t = mp.tile([128, M], f32, tag="am", name="am_t")
                            lm_t = mp.tile([128, M], f32, tag="lm", name="lm_t")
                            nc.sync.dma_start(am_t[:], amh.ap()[g])
                            nc.sync.dma_start(lm_t[:], lmh.ap()[g])
                            t3 = tpool.tile([128, M], f32r, tag="t3", name="t3")
                            nc.vector.scalar_tensor_tensor(
                                t3[:], A[:], -bias_val, lm_t[:], ALU.is_gt, ALU.mult
                            )
                            nc.vector.tensor_tensor(
                                t3[:], t3[:], am_t[:], ALU.add
                            )
                            prev = (B, t3, g)
                    flush_s(prev)

            # ---------------- O phase: E @ v, sums, blend ----------------
            with (
                tc.tile_pool(name="o_v", bufs=3) as vp,
                tc.tile_pool(name="o_x", bufs=1) as xop,
                tc.tile_pool(name="o_out", bufs=4) as opool,
                tc.tile_pool(name="o_ps", bufs=1, space="PSUM") as psO,
            ):
                xn_s = xop.tile([128, MT, D], f32, name="xn_s")
                nc.sync.dma_start(xn_s[:], xn.ap().rearrange("m p d -> p m d"))
                for dh in range(2):
                    O_ps = [
                        psO.tile([128, 512], f32, tag="O", name=f"O{dh}_{mt}", bufs=4)
                        for mt in range(MT)
                    ]
                    if dh == 0:
                        S_ps = [
                            psO.tile([128, 8], f32, tag="Ssum", name=f"Ssum{mt}", bufs=4)
                            for mt in range(MT)
                        ]
                    for j in range(NCORES):
                        vt = vp.tile([128, 4, 512], f32r, tag="v", name="vt")
                        nc.sync.dma_start(
                            vt[:],
                            zv[j][:, :, dh * 512 : (dh + 1) * 512]
                            .rearrange("v p d -> p v d")
                            .bitcast(f32r),
                        )
                        for gi in range(4):
                            g = j * 4 + gi
                            for mt in range(MT):
                                nc.tensor.matmul(
                                    O_ps[mt][:],
                                    E3[:, g, mt * 128 : (mt + 1) * 128],
                                    vt[:, gi, :],
                                    start=(g == 0),
                                    stop=(g == G - 1),
                                )
                                if dh == 0:
                                    nc.tensor.matmul(
                                        S_ps[mt][:],
                                        E3[:, g, mt * 128 : (mt + 1) * 128],
                                        ones_s[:],
                                        start=(g == 0),
                                        stop=(g == G - 1),
                                    )
                    for mt in range(MT):
                        if dh == 0:
                            nc.vector.reciprocal(
                                recip_s[:, mt : mt + 1], S_ps[mt][:, 0:1]
                            )
                            nc.vector.tensor_tensor(
                                s1_s[:, mt : mt + 1],
                                recip_s[:, mt : mt + 1],
                                bnd_s[:, mt : mt + 1],
                                ALU.mult,
                            )
                        xm_t = opool.tile([128, 512], f32, tag="xm", name="xm_t")
                        nc.vector.tensor_scalar(
                            xm_t[:],
                            xn_s[:, mt, dh * 512 : (dh + 1) * 512],
                            omb_s[:, mt : mt + 1],
                            None,
                            ALU.mult,
                        )
                        ot_t = opool.tile([128, 512], f32, tag="ot", name="ot_t")
                        nc.vector.tensor_scalar(
                            ot_t[:],
                            O_ps[mt][:],
                            s1_s[:, mt : mt + 1],
                            None,
                            ALU.mult,
                        )
                        nc.vector.tensor_tensor(
                            ot_t[:], ot_t[:], xm_t[:], ALU.add
                        )
                        nc.sync.dma_start(
                            out.ap()[mt, :, dh * 512 : (dh + 1) * 512], ot_t[:]
                        )

    nc.compile()
    return nc


def make_in_maps(x, attention_mask, learnable_mask, boundary_mask,
                 W_q, b_q, W_k, b_k, W_v, b_v, connection):
    x = np.asarray(x, np.float32)
    amh_full = (np.asarray(attention_mask, np.float32) - 2.0) * MSCALE
    lmh_full = np.asarray(learnable_mask, np.float32) * MSCALE
    boundary = np.asarray(boundary_mask, np.float32).reshape(N)
    wqt_h = np.ascontiguousarray(np.asarray(W_q, np.float32).T).reshape(DC, 128, D)
    wkt_h = np.ascontiguousarray(np.asarray(W_k, np.float32).T).reshape(DC, 128, D)
    wvt_h = np.ascontiguousarray(np.asarray(W_v, np.float32).T).reshape(DC, 128, D)
    cn_h = np.ascontiguousarray(np.asarray(connection, np.float32)).reshape(DC, 128, D)
    bq_h = np.ascontiguousarray(np.asarray(b_q, np.float32).reshape(DC, 128).T)
    bk_h = np.ascontiguousarray(np.asarray(b_k, np.float32).reshape(DC, 128).T)
    bv_h = np.ascontiguousarray(np.asarray(b_v, np.float32).reshape(1, D))
    ident_h = np.eye(128, dtype=np.float32)
    in_maps = []
    for c in range(NCORES):
        rows = slice(c * M, (c + 1) * M)
        in_maps.append(dict(
            xt=np.ascontiguousarray(x[rows].T).reshape(DC, 128, M),
            xn=np.ascontiguousarray(x[rows]).reshape(MT, 128, D),
            wqt=wqt_h, wkt=wkt_h, wvt=wvt_h, cn=cn_h,
            bq=bq_h, bk=bk_h, bv=bv_h,
            bnd=np.ascontiguousarray(boundary[rows]).reshape(MT, 128, 1),
            amh=np.ascontiguousarray(amh_full[rows].T).reshape(G, 128, M),
            lmh=np.ascontiguousarray(lmh_full[rows].T).reshape(G, 128, M),
            ident=ident_h,
            ones8=np.ones((128, 8), dtype=np.float32),
            ones1=np.ones((1, 128), dtype=np.float32),
        ))
    return in_maps


_cache = {}


def kernel(x, attention_mask, learnable_mask, boundary_mask,
           W_q, b_q, W_k, b_k, W_v, b_v, connection, bias):
    bias_val = float(np.asarray(bias).reshape(-1)[0])
    if bias_val not in _cache:
        _cache[bias_val] = build(bias_val)
    nc = _cache[bias_val]
    in_maps = make_in_maps(x, attention_mask, learnable_mask, boundary_mask,
                           W_q, b_q, W_k, b_k, W_v, b_v, connection)
    res = bass_utils.run_bass_kernel_spmd(nc, in_maps, core_ids=list(range(NCORES)))
    outs = [res.results[c]["out"].reshape(M, D) for c in range(NCORES)]
    return np.concatenate(outs, axis=0).astype(np.float32)


# revision 35
# speedup vs baseline: 20273.1316x; 20273.1316x over previous
"""Trainium2 Bass kernel for nn_Attention_75093208203309 (sparse attention).

Contract: kernel(**inputs) takes FULL unsharded inputs (numpy), returns the
FULL [4096, 1024] float32 output. Internally shards query rows across 8
NeuronCores; k/v are computed locally per-core and all-gathered on-device.

Layout strategy (all transposes done host-side in numpy):
  - Per core i (rows = 512*i .. 512*(i+1)):
      qT, qcT   [D, 512]   computed on device from xT shard (f32r matmuls)
      kT_local  [D, 512] -> AllGather -> zk  (kT of all rows)
      v_local   [512, D] -> AllGather -> zv  (v natural layout)
      S.T tiles [mk=128, m=512] = k @ qT : lhsT = kT slice (shared stationary
        operand with conn.T = k @ qcT), rhs = qT / qcT.
      masks pre-scaled host-side so exp needs no row max (class-2 entries
        dominate at +10000); softmax normalization applied after O = E @ v
        using per-partition reciprocal sums.
"""

import contextlib

import numpy as np
import ml_dtypes  # noqa: F401  (np bfloat16 views)

import concourse.bass as bass
import concourse.bacc as bacc
import concourse.mybir as mybir
import concourse.tile as tile
from concourse import bass_utils

f32 = mybir.dt.float32
f32r = mybir.dt.float32r
bf16 = mybir.dt.bfloat16
AF = mybir.ActivationFunctionType
ALU = mybir.AluOpType

NCORES = 8
N, D = 4096, 1024
M = N // NCORES          # 512 rows per core
MT = M // 128            # 4 m-tiles
G = N // 128             # 32 mk-tiles
DC = D // 128            # 8 contraction tiles
MSCALE = 320000.0        # 10000 * 32 (folds softmax scale 1/sqrt(D)=1/32)
RG = [list(range(NCORES))]


def build(bias_val: float, timing_mode: bool = False):
    """timing_mode: single-core variant with zk/zv as ExternalInputs and no
    collectives, for TimelineSim cost-model profiling."""
    nc = bacc.Bacc(None, num_devices=NCORES, debug=False)

    xt = nc.dram_tensor("xt", [DC, 128, M], f32, kind="ExternalInput")
    xn = nc.dram_tensor("xn", [MT, 128, D], f32, kind="ExternalInput")
    wqt = nc.dram_tensor("wqt", [DC, 128, D], f32, kind="ExternalInput")
    wkt = nc.dram_tensor("wkt", [DC, 128, D], f32, kind="ExternalInput")
    wvt = nc.dram_tensor("wvt", [DC, 128, D], f32, kind="ExternalInput")
    cn = nc.dram_tensor("cn", [DC, 128, D], f32, kind="ExternalInput")
    bq = nc.dram_tensor("bq", [128, DC], f32, kind="ExternalInput")
    bk = nc.dram_tensor("bk", [128, DC], f32, kind="ExternalInput")
    bv = nc.dram_tensor("bv", [1, D], f32, kind="ExternalInput")
    bnd = nc.dram_tensor("bnd", [MT, 128, 1], f32, kind="ExternalInput")
    amh = nc.dram_tensor("amh", [G, 128, M], mybir.dt.uint8, kind="ExternalInput")
    lmh = nc.dram_tensor("lmh", [G, 128, M], mybir.dt.uint8, kind="ExternalInput")
    ones8 = nc.dram_tensor("ones8", [128, 8], mybir.dt.bfloat16, kind="ExternalInput")
    ones1 = nc.dram_tensor("ones1", [1, 128], f32, kind="ExternalInput")
    out = nc.dram_tensor("out", [MT, 128, D], f32, kind="ExternalOutput")

    with tile.TileContext(nc) as tc, contextlib.ExitStack() as ST:
        pp = ST.enter_context(tc.tile_pool(name="persist", bufs=1))
        dp = ST.enter_context(tc.tile_pool(name="dram", bufs=1, space="DRAM"))

        E3 = [
            pp.tile([128, M], bf16, tag="E3", name=f"E3_{g}", bufs=G)
            for g in range(G)
        ]
        ones_s = pp.tile([128, 8], bf16, name="ones_s")
        onesk1 = pp.tile([1, 128], f32r, name="onesk1")
        bq_s = pp.tile([128, DC], f32, name="bq_s")
        bk_s = pp.tile([128, DC], f32, name="bk_s")
        bv_s = pp.tile([1, D], f32r, name="bv_s")
        bnd_s = pp.tile([128, MT], f32, name="bnd_s")
        recip_s = pp.tile([128, MT], f32, name="recip_s")
        s1_s = pp.tile([128, MT], f32, name="s1_s")
        omb_s = pp.tile([128, MT], f32, name="omb_s")
        shift_s = pp.tile([128, 1], f32, name="shift_s")
        nc.vector.memset(shift_s[:], -20000.0)

        nc.sync.dma_start(ones_s[:], ones8.ap())
        nc.sync.dma_start(onesk1[:], ones1.ap().bitcast(f32r))
        nc.sync.dma_start(bq_s[:], bq.ap())
        nc.sync.dma_start(bk_s[:], bk.ap())
        nc.sync.dma_start(bv_s[:], bv.ap().bitcast(f32r))
        for mt in range(MT):
            nc.sync.dma_start(bnd_s[:, mt : mt + 1], bnd.ap()[mt])
        nc.vector.tensor_scalar(omb_s[:], bnd_s[:], -1.0, 1.0, ALU.mult, ALU.add)

        kt_loc = dp.tile([DC, 128, M], f32, name="kt_loc")
        v_loc = dp.tile([MT, 128, D], bf16, name="v_loc")
        if timing_mode:
            zk = nc.dram_tensor("zk", [NCORES, DC, 128, M], f32,
                                kind="ExternalInput").ap()
            zv = nc.dram_tensor("zv", [NCORES, MT, 128, D], bf16,
                                kind="ExternalInput").ap()
        else:
            zk = dp.tile([NCORES, DC, 128, M], f32, name="zk")
            zv = dp.tile([NCORES, MT, 128, D], bf16, name="zv")

        # pools whose lifetimes cross phase boundaries, closed manually
        q_stack = contextlib.ExitStack()
        qp = q_stack.enter_context(tc.tile_pool(name="qpool", bufs=1))
        kp = q_stack.enter_context(tc.tile_pool(name="s_kt", bufs=3))
        qt_s = qp.tile([128, DC, M], f32r, name="qt_s")
        qct_s = qp.tile([128, DC, M], f32r, name="qct_s")

        ktb_pre = {}

        def load_ktb(j):
            ktb = kp.tile([128, DC, M], f32r, tag="kt", name="ktb")
            nc.sync.dma_start(
                ktb[:], zk[j].rearrange("t p m -> p t m").bitcast(f32r)
            )
            ktb_pre[j] = ktb
            return ktb

        # ---------------- QKV projections (t-outer) ----------------
        with (
            tc.tile_pool(name="qkv_w", bufs=3) as wp,
            tc.tile_pool(name="qkv_x", bufs=1) as xp,
            tc.tile_pool(name="qkv_sb", bufs=3) as sp,
            tc.tile_pool(name="qkv_ps", bufs=8, space="PSUM") as ps1,
        ):
            xt_s = xp.tile([128, DC, M], f32r, name="xt_s")
            for t in range(DC):
                nc.sync.dma_start(xt_s[:, t, :], xt.ap()[t].bitcast(f32r))

            def proj_half(wdram, half, rhs_tile, psums, name):
                """Load half of a weight matrix (output cols half*512..) and
                run the t-outer matmul block: psums[i] = W.T-half @ rhs."""
                w_h = wp.tile([128, DC, 512], f32r, tag="w", name=f"w_{name}{half}")
                for t in range(DC):
                    nc.sync.dma_start(
                        w_h[:, t, :],
                        wdram.ap()[t][:, half * 512 : (half + 1) * 512]
                        .bitcast(f32r),
                    )
                for t in range(DC):
                    for oi in range(4):
                        nc.tensor.matmul(
                            psums[oi][:],
                            w_h[:, t, oi * 128 : (oi + 1) * 128],
                            rhs_tile[:, t, :],
                            start=(t == 0),
                            stop=(t == DC - 1),
                        )

            # kT first: it feeds the first all-gather.
            for half in range(2):
                kps = [
                    ps1.tile([128, M], f32, tag="ps1", name=f"kps{half}{i}")
                    for i in range(4)
                ]
                proj_half(wkt, half, xt_s, kps, "k")
                for oi in range(4):
                    ot = half * 4 + oi
                    kt_sb = sp.tile([128, M], f32, tag="kvsb", name="kt_sb")
                    nc.scalar.activation(
                        kt_sb[:], kps[oi][:], AF.Identity,
                        bias=bk_s[:, ot : ot + 1],
                    )
                    kb = kt_sb[:].bitcast(mybir.dt.uint32)
                    nc.vector.tensor_scalar(kb, kb, 0x800, None, ALU.add)
                    nc.sync.dma_start(kt_loc[ot], kt_sb[:])
            if not timing_mode:
                nc.gpsimd.collective_compute(
                    "AllGather", ALU.bypass, replica_groups=RG,
                    ins=[kt_loc[:].opt()], outs=[zk[:].opt()],
                )
            load_ktb(0)

            # v: halves are the d-halves directly
            for dh in range(2):
                wv_h = wp.tile([128, DC, 512], f32r, tag="w", name=f"w_v{dh}")
                for t in range(DC):
                    nc.sync.dma_start(
                        wv_h[:, t, :],
                        wvt.ap()[t][:, dh * 512 : (dh + 1) * 512].bitcast(f32r),
                    )
                vps = [
                    ps1.tile([128, 512], f32, tag="ps1", name=f"vps{dh}{mt}")
                    for mt in range(MT)
                ]
                for t in range(DC):
                    for mt in range(MT):
                        nc.tensor.matmul(
                            vps[mt][:],
                            xt_s[:, t, mt * 128 : (mt + 1) * 128],
                            wv_h[:, t, :],
                            start=(t == 0),
                            stop=False,
                        )
                for mt in range(MT):
                    nc.tensor.matmul(
                        vps[mt][:],
                        onesk1[:, :],
                        bv_s[:, dh * 512 : (dh + 1) * 512],
                        start=False,
                        stop=True,
                    )
                    v_sb = sp.tile([128, 512], bf16, tag="kvsb", name="v_sb")
                    nc.scalar.copy(v_sb[:], vps[mt][:])
                    nc.sync.dma_start(
                        v_loc[mt, :, dh * 512 : (dh + 1) * 512], v_sb[:]
                    )
            if not timing_mode:
                nc.gpsimd.collective_compute(
                    "AllGather", ALU.bypass, replica_groups=RG,
                    ins=[v_loc[:].opt()], outs=[zv[:].opt()],
                )

            for half in range(2):
                qps = [
                    ps1.tile([128, M], f32, tag="ps1", name=f"qps{half}{i}")
                    for i in range(4)
                ]
                proj_half(wqt, half, xt_s, qps, "q")
                for oi in range(4):
                    ot = half * 4 + oi
                    qtmp = sp.tile([128, M], f32, tag="kvsb", name="qtmp")
                    nc.scalar.activation(
                        qtmp[:], qps[oi][:], AF.Identity,
                        bias=bq_s[:, ot : ot + 1],
                    )
                    qb = qtmp[:].bitcast(mybir.dt.uint32)
                    nc.vector.tensor_scalar(qb, qb, 0x800, None, ALU.add)
                    nc.vector.tensor_copy(qt_s[:, ot, :], qtmp[:])

            for half in range(2):
                cps = [
                    ps1.tile([128, M], f32, tag="ps1", name=f"cps{half}{i}")
                    for i in range(4)
                ]
                proj_half(cn, half, qt_s, cps, "c")
                for oi in range(4):
                    ot = half * 4 + oi
                    ctmp = sp.tile([128, M], f32, tag="kvsb", name="ctmp")
                    nc.scalar.copy(ctmp[:], cps[oi][:])
                    cb = ctmp[:].bitcast(mybir.dt.uint32)
                    nc.vector.tensor_scalar(cb, cb, 0x800, None, ALU.add)
                    nc.vector.tensor_copy(qct_s[:, ot, :], ctmp[:])

        # v tiles + xn survive into the O phase
        o_stack = contextlib.ExitStack()
        vpool = o_stack.enter_context(tc.tile_pool(name="o_v", bufs=3, side="right"))
        xop = o_stack.enter_context(tc.tile_pool(name="o_x", bufs=1, side="right"))
        xn_s = xop.tile([128, MT, D], f32, name="xn_s")
        nc.sync.dma_start(xn_s[:], xn.ap().rearrange("m p d -> p m d"))
        vt_pre = {}

        def load_vt(dh, j):
            vt = vpool.tile([128, 4, 512], bf16, tag="v", name="vt", bufs=4)
            for vb in range(4):
                nc.sync.dma_start(
                    vt[:, vb, :], zv[j][vb][:, dh * 512 : (dh + 1) * 512]
                )
            vt_pre[(dh, j)] = vt
            return vt

        # ---------------- S phase: logits, mask, exp ----------------
        with (
            tc.tile_pool(name="s_m", bufs=12) as mp,
            tc.tile_pool(name="s_t", bufs=4) as tpool,
            tc.tile_pool(name="s_psA", bufs=3, space="PSUM") as psA,
            tc.tile_pool(name="s_psB", bufs=3, space="PSUM") as psB,
        ):
            load_vt(0, 0)
            for j in range(NCORES):
                ktb = ktb_pre.pop(j, None) or load_ktb(j)
                ktb_pre.pop(j, None)
                for gi in range(4):
                    g = j * 4 + gi
                    B = psB.tile([128, M], f32, tag="B", name="Bps")
                    for t in range(DC):
                        nc.tensor.matmul(
                            B[:],
                            ktb[:, t, gi * 128 : (gi + 1) * 128],
                            qt_s[:, t, :],
                            start=(t == 0),
                            stop=(t == DC - 1),
                        )
                    A = psA.tile([128, M], f32, tag="A", name="Aps")
                    for t in range(DC):
                        nc.tensor.matmul(
                            A[:],
                            ktb[:, t, gi * 128 : (gi + 1) * 128],
                            qct_s[:, t, :],
                            start=(t == 0),
                            stop=(t == DC - 1),
                        )
                    am_t = mp.tile([128, M], mybir.dt.uint8, tag="am", name="am_t")
                    lm_t = mp.tile([128, M], mybir.dt.uint8, tag="lm", name="lm_t")
                    nc.sync.dma_start(am_t[:], amh.ap()[g])
                    nc.sync.dma_start(lm_t[:], lmh.ap()[g])
                    t3 = tpool.tile([128, M], f32, tag="t3", name="t3")
                    nc.vector.scalar_tensor_tensor(
                        t3[:], A[:], -bias_val, lm_t[:], ALU.is_gt, ALU.mult
                    )
                    nc.vector.tensor_tensor(t3[:], t3[:], am_t[:], ALU.add)
                    comb = tpool.tile([128, M], f32, tag="comb", name="comb")
                    nc.vector.scalar_tensor_tensor(
                        comb[:], t3[:], MSCALE, B[:], ALU.mult, ALU.add
                    )
                    # -20000 = the (am+st*lm-2) shift, folded into the exp bias
                    nc.scalar.activation(
                        E3[g][:], comb[:], AF.Exp, scale=1.0 / 32.0, bias=shift_s[:, 0:1]
                    )
        q_stack.close()  # qt/qct + ktb SBUF released before O phase

        # ---------------- O phase: E @ v, sums, blend ----------------
        with (
            tc.tile_pool(name="o_out", bufs=4) as opool,
            tc.tile_pool(name="o_xm", bufs=1) as xmp,
            tc.tile_pool(name="o_ps", bufs=1, space="PSUM") as psO,
        ):
            # xm = (1-boundary)*x for all 8 chunks, off the critical path
            xm_t = {}
            for mt in range(MT):
                for dh in range(2):
                    xm = xmp.tile([128, 512], f32, name=f"xm_{mt}_{dh}")
                    nc.vector.tensor_scalar(
                        xm[:],
                        xn_s[:, mt, dh * 512 : (dh + 1) * 512],
                        omb_s[:, mt : mt + 1],
                        None,
                        ALU.mult,
                    )
                    xm_t[(mt, dh)] = xm
            for dh in range(2):
                O_ps = [
                    psO.tile([128, 512], f32, tag="O", name=f"O{dh}_{mt}", bufs=4)
                    for mt in range(MT)
                ]
                if dh == 0:
                    S_ps = [
                        psO.tile([128, 8], f32, tag="Ssum", name=f"S{mt}", bufs=4)
                        for mt in range(MT)
                    ]
                for j in range(NCORES):
                    vt = vt_pre.pop((dh, j), None) or load_vt(dh, j)
                    vt_pre.pop((dh, j), None)
                    for gi in range(4):
                        g = j * 4 + gi
                        for mt in range(MT):
                            nc.tensor.matmul(
                                O_ps[mt][:],
                                E3[g][:, mt * 128 : (mt + 1) * 128],
                                vt[:, gi, :],
                                start=(g == 0),
                                stop=(g == G - 1),
                            )
                            if dh == 0:
                                nc.tensor.matmul(
                                    S_ps[mt][:],
                                    E3[g][:, mt * 128 : (mt + 1) * 128],
                                    ones_s[:],
                                    start=(g == 0),
                                    stop=(g == G - 1),
                                )
                for mt in range(MT):
                    if dh == 0:
                        nc.vector.reciprocal(
                            recip_s[:, mt : mt + 1], S_ps[mt][:, 0:1]
                        )
                        nc.vector.tensor_tensor(
                            s1_s[:, mt : mt + 1],
                            recip_s[:, mt : mt + 1],
                            bnd_s[:, mt : mt + 1],
                            ALU.mult,
                        )
                    # normalize + boundary-scale on ACT (PSUM read, scale AP)
                    ot_t = opool.tile([128, 512], f32, tag="ot", name="ot_t")
                    nc.scalar.mul(ot_t[:], O_ps[mt][:], s1_s[:, mt : mt + 1])
                    nc.vector.tensor_tensor(
                        ot_t[:], ot_t[:], xm_t[(mt, dh)][:], ALU.add
                    )
                    nc.sync.dma_start(
                        out.ap()[mt, :, dh * 512 : (dh + 1) * 512], ot_t[:]
                    )
        o_stack.close()

    nc.compile()
    return nc


def _rne11(a):
    """Round float32 array to 11 mantissa bits, round-half-to-even."""
    u = np.ascontiguousarray(a, np.float32).view(np.uint32)
    lsb = (u >> np.uint32(12)) & np.uint32(1)
    u2 = ((u + np.uint32(0x7FF) + lsb) >> np.uint32(12)) << np.uint32(12)
    return u2.view(np.float32)


def make_in_maps(x, attention_mask, learnable_mask, boundary_mask,
                 W_q, b_q, W_k, b_k, W_v, b_v, connection):
    x = np.asarray(x, np.float32)
    x11 = _rne11(x)
    amh_full = np.asarray(attention_mask, np.float32).astype(np.uint8)
    lmh_full = np.asarray(learnable_mask, np.float32).astype(np.uint8)
    boundary = np.asarray(boundary_mask, np.float32).reshape(N)
    wqt_h = np.ascontiguousarray(_rne11(np.asarray(W_q, np.float32).T)).reshape(DC, 128, D)
    wkt_h = np.ascontiguousarray(_rne11(np.asarray(W_k, np.float32).T)).reshape(DC, 128, D)
    wvt_h = np.ascontiguousarray(_rne11(np.asarray(W_v, np.float32).T)).reshape(DC, 128, D)
    cn_h = np.ascontiguousarray(_rne11(np.asarray(connection, np.float32))).reshape(DC, 128, D)
    bq_h = np.ascontiguousarray(np.asarray(b_q, np.float32).reshape(DC, 128).T)
    bk_h = np.ascontiguousarray(np.asarray(b_k, np.float32).reshape(DC, 128).T)
    bv_h = np.ascontiguousarray(_rne11(np.asarray(b_v, np.float32).reshape(1, D)))
    in_maps = []
    for c in range(NCORES):
        rows = slice(c * M, (c + 1) * M)
        in_maps.append(dict(
            xt=np.ascontiguousarray(x11[rows].T).reshape(DC, 128, M),
            xn=np.ascontiguousarray(x[rows]).reshape(MT, 128, D),
            wqt=wqt_h, wkt=wkt_h, wvt=wvt_h, cn=cn_h,
            bq=bq_h, bk=bk_h, bv=bv_h,
            bnd=np.ascontiguousarray(boundary[rows]).reshape(MT, 128, 1),
            amh=np.ascontiguousarray(amh_full[rows].T).reshape(G, 128, M),
            lmh=np.ascontiguousarray(lmh_full[rows].T).reshape(G, 128, M),
            ones8=np.ones((128, 8), dtype=ml_dtypes.bfloat16),
            ones1=np.ones((1, 128), dtype=np.float32),
        ))
    return in_maps


_cache = {}


def kernel(x, attention_mask, learnable_mask, boundary_mask,
           W_q, b_q, W_k, b_k, W_v, b_v, connection, bias):
    bias_val = float(np.asarray(bias).reshape(-1)[0])
    if bias_val not in _cache:
        _cache[bias_val] = build(bias_val)
    nc = _cache[bias_val]
    in_maps = make_in_maps(x, attention_mask, learnable_mask, boundary_mask,
                           W_q, b_q, W_k, b_k, W_v, b_v, connection)
    res = bass_utils.run_bass_kernel_spmd(nc, in_maps, core_ids=list(range(NCORES)))
    outs = [res.results[c]["out"].reshape(M, D) for c in range(NCORES)]
    return np.concatenate(outs, axis=0).astype(np.float32)
